# revision 1
# baseline (speedup 1.0000x reference)
"""LIF spiking-neuron kernel for Trainium2 (Bass/Tile), 8-core SPMD.

Problem: x [B=32, T=8, C=128, H=32, W=32] f32.  Per (b,c,h,w) neuron,
sequential over T:
    mem = mem*TAU + x_t;  spike = (mem - 1 > 0);  mem = 0 if spike
TAU = 0.5, THRESH = 1.0.

Sharding: batch dim B=32 split 4-per-core across 8 NeuronCores; the
recurrence is along T only, so there is no communication.

Per-core algorithm (bit-exact vs the fp32 reference):
  TAU = 0.5 is a power of two, so rescale the state M_t = 2^t * m_t.
  The decay becomes a pure add:  M_t = M_{t-1} + 2^t * x_t  (the 2^t
  prescale of x is exact in fp32, and power-of-2 scaling commutes with
  fp rounding, so every M_t is bit-exactly 2^t * m_t).
  spike_t = (M_t > 2^t)  <=>  (m_t > 1)  <=>  reference's (m_t - 1 > 0).
  Reset: M <- 0 where spike (copy_predicated with a zeros tile).

Engine split per (b,t) tile of [C=128 partitions, H*W=1024]:
  ACT:  y_t = 2^t * x_t  (prescale, off the recurrence critical path)
  DVE:  M += y_t ; spike = M > 2^t ; M <- 0 where spike
  SP :  in/out DMAs (16 HW DMA engines, ~360GB/s aggregate)
"""

import numpy as np

from concourse import bacc, bass, mybir, tile
from concourse.alu_op_type import AluOpType
from concourse.bass_utils import run_bass_kernel_spmd

# Full-problem shape (hardcoded per harness contract).
B, T, C, H, W = 32, 8, 128, 32, 32
N_CORES = 8
B_LOC = B // N_CORES          # 4 batches per core
F = H * W                     # 1024 free elements per tile
FP32 = mybir.dt.float32

_NC_CACHE = {}


PAIR = 2                      # batches fused per tile
G = B_LOC // PAIR             # chain groups per core
FW = PAIR * F                 # 2048 free elements per tile


def _emit(tc, x_d, o_d):
    nc = tc.nc
    # per (group, t) DRAM view: [c, pair, h*w] — 2 batches fused per tile
    def dram3(ap, g, t):
        return ap[g * PAIR : (g + 1) * PAIR, t].rearrange("p c h w -> c p (h w)")

    with (
        tc.tile_pool(name="xp", bufs=12) as xp,
        tc.tile_pool(name="sp", bufs=6) as sp,
        tc.tile_pool(name="gp", bufs=3) as gp,
        tc.tile_pool(name="mp", bufs=G) as mp,
        tc.tile_pool(name="zp", bufs=1) as zp,
    ):
        z = zp.tile([C, FW], FP32)
        nc.scalar.memzero(z)

        # per-t [128,1] bias columns holding -2^t for the ACT Sign compare
        biases = []
        for t in range(T):
            bt = zp.tile([C, 1], FP32, name=f"bias{t}")
            nc.gpsimd.memset(bt, -float(2.0**t))
            biases.append(bt)

        ms = [mp.tile([C, FW], FP32, name=f"m{g}") for g in range(G)]
        # t-major emission interleaves the two chains; chain 0's spike
        # compare runs on ACT (Relu(Sign(M - theta)), exact 0/1), chain 1's
        # on DVE (is_gt, 2x perf mode) to balance engine load.
        for t in range(T):
            th = float(2.0**t)
            for g in range(G):
                m = ms[g]
                xt = xp.tile([C, FW], FP32)
                # alternate input DMAs across SP/ACT HWDGE queues so DGE
                # setup of one overlaps the other's transfer
                dma_eng = nc.sync if (t * G + g) % 2 == 0 else nc.scalar
                dma_eng.dma_start(
                    out=xt.rearrange("c (p f) -> c p f", p=PAIR),
                    in_=dram3(x_d, g, t),
                )
                if t == 0:
                    nc.scalar.copy(m, xt)                       # M = x_0 (ACT)
                else:
                    # fused prescale+accumulate: M = (x_t * 2^t) + M
                    nc.vector.scalar_tensor_tensor(
                        m, xt, th, m, AluOpType.mult, AluOpType.add
                    )
                # spike mask as u8: chain 0 computes it on ACT
                # (Relu(Sign(M - theta)) cast to u8 — the cast maps Relu's
                # -0.0 to integer 0), chain 1 on DVE (is_gt, 2x perf mode).
                s = sp.tile([C, FW], mybir.dt.uint8)
                if g % 2 == 0:
                    sg = gp.tile([C, FW], FP32)
                    nc.scalar.activation(
                        sg, m, mybir.ActivationFunctionType.Sign, bias=biases[t]
                    )
                    nc.scalar.activation(s, sg, mybir.ActivationFunctionType.Relu)
                else:
                    nc.vector.tensor_single_scalar(s, m, th, AluOpType.is_gt)
                # f32 0/1 DRAM output via gpsimd cast-DMA (SWDGE converts
                # u8 -> f32 in flight; no extra DVE/ACT pass)
                nc.gpsimd.dma_start(
                    out=dram3(o_d, g, t),
                    in_=s.rearrange("c (p f) -> c p f", p=PAIR),
                )
                if t < T - 1:
                    nc.vector.copy_predicated(m, s, z)


def build_nc():
    """Build + compile the per-core Bass program (cached)."""
    if "nc" in _NC_CACHE:
        return _NC_CACHE["nc"]
    nc = bacc.Bacc(
        "TRN2",
        target_bir_lowering=False,
        debug=False,
        enable_asserts=False,
        num_devices=N_CORES,
    )
    x_d = nc.dram_tensor("x", [B_LOC, T, C, H, W], FP32, kind="ExternalInput").ap()
    o_d = nc.dram_tensor("out", [B_LOC, T, C, H, W], FP32, kind="ExternalOutput").ap()
    with tile.TileContext(nc) as tc:
        _emit(tc, x_d, o_d)
    nc.compile()
    _NC_CACHE["nc"] = nc
    return nc


def make_in_maps(x: np.ndarray) -> list[dict[str, np.ndarray]]:
    assert x.shape == (B, T, C, H, W) and x.dtype == np.float32, (x.shape, x.dtype)
    return [
        {"x": np.ascontiguousarray(x[i * B_LOC : (i + 1) * B_LOC])}
        for i in range(N_CORES)
    ]


def kernel(x: np.ndarray) -> np.ndarray:
    x = np.asarray(x, dtype=np.float32)
    nc = build_nc()
    res = run_bass_kernel_spmd(nc, make_in_maps(x), list(range(N_CORES)))
    return np.concatenate([r["out"] for r in res.results], axis=0)

